# revision 1
# baseline (speedup 1.0000x reference)
"""Trainium2 Bass kernel for CausalSelfAttention with KV-prefix cache.

Problem (hardcoded): B=2, T=2048, C=1024, H=16, D=64, P=2048.
Sharding: 8 cores = 2 (batch) x 4 (head groups of 4 heads).
Each core computes, for its (b, 4 heads):
  qkv slice -> prefix+causal softmax -> AV -> partial W_proj product.
Host sums the 4 per-core partial projections per batch and transposes.

All attention math runs in a transposed layout (features/keys on the
partition dim) so no on-device transposes are ever needed:
  x^T [C,T] -> q^T,k^T (pair-packed [128,T]) via lhsT=W_attn slice
  S^T [keys, tq] via row-tiled (2 heads) K=64 matmuls
  exp on ScalarE (PSUM->SBUF, bf16, fused scale=1/sqrt(D))
  AV + denom via col-tiled matmuls accumulated in PSUM over key chunks
  y^T / denom via gpsimd partition-broadcast + DVE multiply
  out^T = W_proj_slice^T @ y^T  (per-core partial)
"""

import numpy as np
import ml_dtypes
from contextlib import ExitStack

import concourse.bacc as bacc
import concourse.tile as tile
import concourse.mybir as mybir
from concourse.bass_utils import run_bass_kernel_spmd

F32 = mybir.dt.float32
F32R = mybir.dt.float32r
BF16 = mybir.dt.bfloat16
EXP = mybir.ActivationFunctionType.Exp
COPY = mybir.ActivationFunctionType.Copy
MULT = mybir.AluOpType.mult

B, T, C, H, D, P = 2, 2048, 1024, 16, 64, 2048
HPC = 4            # heads per core
NPAIR = 2          # head pairs per core
TQ = 512           # query block (matmul free dim)
KC = 128           # key chunk (PSUM partition dim)
G = 3              # key chunks per exp group (6 PSUM banks with 2 heads)
SCALE = 1.0 / np.sqrt(D)


def build_kernel(t=T, p=P, c=C, n_cores=8, dbg=False, phases=(1, 2, 3), p2sub=7):
    """Build + compile the SPMD Bass program. Same program on every core."""
    nt = t // TQ            # query blocks
    npc = p // KC           # prefix key chunks
    nck = c // 128          # C contraction chunks
    ntc = t // 128          # T rows in 128-chunks

    nc = bacc.Bacc("TRN2", target_bir_lowering=False, debug=False,
                   num_devices=n_cores)
    dbg_t = {}
    if dbg:
        for nm, shp in [("dbg_qT", [128, 512]), ("dbg_kT", [128, 512]),
                        ("dbg_vt", [128, 256]), ("dbg_eb", [128, 2 * G, TQ]),
                        ("dbg_db", [128, TQ]), ("dbg_bc", [128, TQ]),
                        ("dbg_y", [128, TQ]), ("dbg_yu", [128, TQ])]:
            dt_ = BF16 if nm in ("dbg_eb", "dbg_vt") else F32
            dbg_t[nm] = nc.dram_tensor(nm, shp, dt_, kind="ExternalOutput").ap()

    x_t = nc.dram_tensor("x_t", [c, t], F32, kind="ExternalInput").ap()
    w_qk = nc.dram_tensor("w_qk", [c, 2 * HPC * D], F32, kind="ExternalInput").ap()
    w_v = nc.dram_tensor("w_v", [c, HPC * D], F32, kind="ExternalInput").ap()
    ckt = nc.dram_tensor("ckt", [NPAIR, 128, p], F32, kind="ExternalInput").ap()
    cv = nc.dram_tensor("cv", [NPAIR, 128, npc, 128], BF16, kind="ExternalInput").ap()
    wp = nc.dram_tensor("wp", [NPAIR, 128, c], F32, kind="ExternalInput").ap()
    masks = nc.dram_tensor("masks", [128, 4, TQ], BF16, kind="ExternalInput").ap()
    ones = nc.dram_tensor("ones", [128, 1], BF16, kind="ExternalInput").ap()
    bsel = nc.dram_tensor("bsel", [33, 128], F32, kind="ExternalInput").ap()
    zrd = nc.dram_tensor("zrd", [33, TQ], F32, kind="ExternalInput").ap()
    out_t = nc.dram_tensor("out_t", [c, t], F32, kind="ExternalOutput").ap()

    with tile.TileContext(nc) as tc, ExitStack() as top:
        const = top.enter_context(tc.tile_pool(name="const", bufs=1))
        persist = top.enter_context(tc.tile_pool(name="persist", bufs=1))

        # ---- persistent SBUF tensors -------------------------------------
        qT = [persist.tile([128, t], F32R, tag=f"qT{i}", name=f"qT{i}") for i in range(NPAIR)]
        kT = [persist.tile([128, t], F32R, tag=f"kT{i}", name=f"kT{i}") for i in range(NPAIR)]
        cktT = [persist.tile([128, p], F32R, tag=f"cktT{i}", name=f"cktT{i}") for i in range(NPAIR)]
        vt = persist.tile([128, ntc, 2 * 128], BF16, tag="vt", name="vt")
        cvt = [persist.tile([128, npc, 128], BF16, tag=f"cvt{i}", name=f"cvt{i}") for i in range(NPAIR)]
        wpt = [persist.tile([128, c], F32R, tag=f"wpt{i}", name=f"wpt{i}") for i in range(NPAIR)]
        maskt = const.tile([128, 4, TQ], BF16, tag="maskt", name="maskt")
        onest = const.tile([128, 1], BF16, tag="onest", name="onest")
        bselt = const.tile([33, 128], F32R, tag="bselt", name="bselt")
        ysb = [persist.tile([128, nt, TQ], F32R, tag=f"ysb{i}", name=f"ysb{i}") for i in range(NPAIR)]
        rd = persist.tile([33, TQ], F32R, tag="rd", name="rd")
        nc.sync.dma_start(rd[:], zrd[:, :].bitcast(F32R))

        nc.sync.dma_start(maskt[:], masks[:, :, :])
        nc.sync.dma_start(onest[:], ones[:, :])
        nc.sync.dma_start(bselt[:], bsel[:, :].bitcast(F32R))
        for i in range(NPAIR):
            nc.sync.dma_start(cktT[i][:], ckt[i, :, :].bitcast(F32R))
            nc.sync.dma_start(cvt[i][:], cv[i, :, :, :])
            nc.sync.dma_start(wpt[i][:], wp[i, :, :].bitcast(F32R))

        # ---- phase 1: QKV projection -------------------------------------
        with ExitStack() as ph1:
          if 1 in phases:
              qkv_in = ph1.enter_context(tc.tile_pool(name="qkv_in", bufs=1))
              ps_qk = ph1.enter_context(tc.tile_pool(name="ps_qk", bufs=4, space="PSUM"))
              ps_v = ph1.enter_context(tc.tile_pool(name="ps_v", bufs=4, space="PSUM"))

              xt = qkv_in.tile([128, nck, t], F32R, tag="xt", name="xt")
              wqkt = qkv_in.tile([128, nck, 2 * HPC * D], F32R, tag="wqkt", name="wqkt")
              wvt = qkv_in.tile([128, nck, HPC * D], F32R, tag="wvt", name="wvt")
              for kc_ in range(nck):
                  nc.sync.dma_start(xt[:, kc_, :], x_t[kc_ * 128:(kc_ + 1) * 128, :].bitcast(F32R))
                  nc.sync.dma_start(wqkt[:, kc_, :], w_qk[kc_ * 128:(kc_ + 1) * 128, :].bitcast(F32R))
                  nc.sync.dma_start(wvt[:, kc_, :], w_v[kc_ * 128:(kc_ + 1) * 128, :].bitcast(F32R))

              # q^T / k^T: out chunk mc (128 rows = one head pair of q or k)
              for mc in range(4):
                  dest = qT[mc] if mc < 2 else kT[mc - 2]
                  for nb in range(nt):
                      ps = ps_qk.tile([128, TQ], F32, tag="ps_qk", name="psqk")
                      for kc_ in range(nck):
                          nc.tensor.matmul(
                              ps[:],
                              wqkt[:, kc_, mc * 128:(mc + 1) * 128],
                              xt[:, kc_, nb * TQ:(nb + 1) * TQ],
                              start=(kc_ == 0), stop=(kc_ == nck - 1),
                          )
                      nc.scalar.activation(dest[:, nb * TQ:(nb + 1) * TQ], ps[:], COPY)

              # v in natural layout [t, 256]
              for tc_ in range(ntc):
                  ps = ps_v.tile([128, HPC * D], F32, tag="ps_v", name="psv")
                  for kc_ in range(nck):
                      nc.tensor.matmul(
                          ps[:],
                          xt[:, kc_, tc_ * 128:(tc_ + 1) * 128],
                          wvt[:, kc_, :],
                          start=(kc_ == 0), stop=(kc_ == nck - 1),
                      )
                  nc.scalar.activation(vt[:, tc_, :], ps[:], COPY)

        if dbg:
            nc.sync.dma_start(dbg_t["dbg_qT"][:, :], qT[0][:, 0:512].bitcast(F32))
            nc.sync.dma_start(dbg_t["dbg_kT"][:, :], kT[0][:, 0:512].bitcast(F32))
            nc.sync.dma_start(dbg_t["dbg_vt"][:, :], vt[:, 0, :])

        # ---- phase 2: attention ------------------------------------------
        with ExitStack() as ph2:
          if 2 in phases:
              ps_s = ph2.enter_context(tc.tile_pool(name="ps_s", bufs=1, space="PSUM"))
              ps_y = ph2.enter_context(tc.tile_pool(name="ps_y", bufs=1, space="PSUM"))
              ps_d = ph2.enter_context(tc.tile_pool(name="ps_d", bufs=1, space="PSUM"))
              expp = ph2.enter_context(tc.tile_pool(name="expp", bufs=2))
              nrm = ph2.enter_context(tc.tile_pool(name="nrm", bufs=2))

              for pair in range(NPAIR):
                  for tb in range(nt):
                      ncur = (tb + 1) * (TQ // KC)   # current-key chunks
                      nkc = npc + ncur               # total key chunks
                      yb = ps_y.tile([128, TQ], F32, tag="yb", name="yb")
                      db = ps_d.tile([128, TQ], F32, tag="db", name="db")
                      first, last = 0, nkc - 1
                      g0 = 0
                      while g0 < nkc:
                          gc = min(G, nkc - g0)
                          sb = ps_s.tile([128, 2 * G, TQ], F32, tag="sb", name="sb")
                          # scores S^T for both heads (row-tiled K=64 pairs)
                          for i in range(gc):
                              kc_ = g0 + i
                              if kc_ < npc:
                                  ksrc = cktT[pair]
                                  klo = kc_ * KC
                              else:
                                  ksrc = kT[pair]
                                  klo = (kc_ - npc) * KC
                              for h in range(2):
                                  nc.tensor.matmul(
                                      sb[:, h * G + i, :],
                                      ksrc[h * 64:(h + 1) * 64, klo:klo + KC],
                                      qT[pair][h * 64:(h + 1) * 64, tb * TQ:(tb + 1) * TQ],
                                      start=True, stop=True,
                                      tile_position=(h * 64, 0),
                                      skip_group_check=True,
                                  )
                          # fused exp over the group's banks -> bf16 SBUF
                          eb = expp.tile([128, 2 * G, TQ], BF16, tag="eb", name="eb")
                          if gc == G:
                              nc.scalar.activation(eb[:], sb[:], EXP, scale=SCALE)
                          else:
                              for h in range(2):
                                  nc.scalar.activation(
                                      eb[:, h * G:h * G + gc, :],
                                      sb[:, h * G:h * G + gc, :],
                                      EXP, scale=SCALE)
                          # causal masks on diagonal chunks
                          for i in range(gc if p2sub >= 2 else 0):
                              kc_ = g0 + i
                              j = kc_ - (nkc - 4)
                              if j >= 0:
                                  for h in range(2):
                                      nc.vector.tensor_tensor(
                                          eb[:, h * G + i, :], eb[:, h * G + i, :],
                                          maskt[:, j, :], MULT)
                          # AV (col-tiled pair) + denominators, PSUM-accumulated
                          for i in range(gc if p2sub >= 3 else 0):
                              kc_ = g0 + i
                              if kc_ < npc:
                                  vsrc_e = cvt[pair][:, kc_, 0:64]
                                  vsrc_o = cvt[pair][:, kc_, 64:128]
                              else:
                                  ck = kc_ - npc
                                  vsrc_e = vt[:, ck, pair * 128:pair * 128 + 64]
                                  vsrc_o = vt[:, ck, pair * 128 + 64:pair * 128 + 128]
                              st = (kc_ == first)
                              sp = (kc_ == last)
                              nc.tensor.matmul(yb[0:64, :], vsrc_e, eb[:, i, :],
                                               start=st, stop=sp,
                                               tile_position=(0, 0),
                                               skip_group_check=True)
                              nc.tensor.matmul(yb[64:128, :], vsrc_o, eb[:, G + i, :],
                                               start=st, stop=sp,
                                               tile_position=(0, 64),
                                               skip_group_check=True)
                              if p2sub < 4:
                                  continue
                              nc.tensor.matmul(db[0:1, :], onest[:], eb[:, i, :],
                                               start=st, stop=sp,
                                               tile_position=(0, 0),
                                               skip_group_check=True)
                              nc.tensor.matmul(db[32:33, :], onest[:], eb[:, G + i, :],
                                               start=st, stop=sp,
                                               tile_position=(0, 32),
                                               skip_group_check=True)
                          if dbg and pair == 0 and tb == 0 and g0 == 0:
                              nc.sync.dma_start(dbg_t["dbg_eb"][:, :, :], eb[:])
                          g0 += gc

                      # evacuate unnormalized y^T (releases the PSUM bank fast)
                      if p2sub < 3:
                          continue
                      ysl = ysb[pair][:, tb, :]
                      nc.vector.tensor_copy(ysl, yb[:])
                      # reciprocal of denominators, broadcast via DRAM bounce
                      if p2sub < 5:
                          continue
                      with nc.allow_low_precision(reason="recip->f32r for bcast mm"):
                          nc.vector.reciprocal(rd[0:1, :], db[0:1, :])
                          nc.vector.reciprocal(rd[32:33, :], db[32:33, :])
                      # broadcast recips across partitions via K=33 matmul
                      # (bsel rows other than 0/32 are zero -> garbage killed)
                      bcp = ps_d.tile([128, TQ], F32, tag="db", name="bcp")
                      if p2sub >= 6:
                          nc.tensor.matmul(bcp[:], bselt[:], rd[:],
                                           start=True, stop=True,
                                           skip_group_check=True)
                      if dbg and pair == 0 and tb == 0:
                          nc.sync.dma_start(dbg_t["dbg_yu"][:, :], ysl.bitcast(F32))
                          nc.sync.dma_start(dbg_t["dbg_db"][0:33, :], rd[:].bitcast(F32))
                      # in-place normalize in SBUF (off the PSUM critical path)
                      if p2sub >= 7:
                          nc.vector.tensor_tensor(ysl, ysl, bcp[:], MULT)
                      if dbg and pair == 0 and tb == 0:
                          bstg = nrm.tile([128, TQ], F32, tag="bstg", name="bstg")
                          nc.scalar.activation(bstg[:], bcp[:], COPY)
                          nc.sync.dma_start(dbg_t["dbg_bc"][:, :], bstg[:])
                      if dbg and pair == 0 and tb == 0:
                          nc.sync.dma_start(dbg_t["dbg_y"][:, :],
                                            ysb[0][:, 0, :].bitcast(F32))

        # ---- phase 3: output projection ----------------------------------
        with ExitStack() as ph3:
          if 3 in phases:
              ps_o = ph3.enter_context(tc.tile_pool(name="ps_o", bufs=4, space="PSUM"))
              stg = ph3.enter_context(tc.tile_pool(name="stg", bufs=4))
              for mc in range(c // 128):
                  for nb in range(nt):
                      ps = ps_o.tile([128, TQ], F32, tag="ps_o", name="pso")
                      for pair in range(NPAIR):
                          nc.tensor.matmul(
                              ps[:],
                              wpt[pair][:, mc * 128:(mc + 1) * 128],
                              ysb[pair][:, nb, :],
                              start=(pair == 0), stop=(pair == NPAIR - 1),
                          )
                      ot = stg.tile([128, TQ], F32, tag="ot", name="ot")
                      nc.scalar.activation(ot[:], ps[:], COPY)
                      nc.sync.dma_start(
                          out_t[mc * 128:(mc + 1) * 128, nb * TQ:(nb + 1) * TQ], ot[:])

    nc.compile()
    return nc


def make_in_maps(x, W_attn, W_proj, cache_k, cache_v, n_cores=8):
    """Shard full inputs into per-core input maps (host side)."""
    b_, t_, c_ = x.shape
    h_ = cache_k.shape[1]
    d_ = c_ // h_
    p_ = cache_k.shape[2]
    hpc = h_ // (n_cores // b_)
    in_maps = []
    Wq = W_attn[:, 0 * c_:1 * c_]
    Wk = W_attn[:, 1 * c_:2 * c_]
    Wv = W_attn[:, 2 * c_:3 * c_]
    mask_np = np.zeros((128, 4, TQ), np.float32)
    for j in range(4):
        mask_np[:, j, :] = (np.arange(TQ)[None, :] >=
                            (np.arange(128)[:, None] + j * 128)).astype(np.float32)
    ones_np = np.ones((128, 1), np.float32)
    bsel_np = np.zeros((33, 128), np.float32)
    bsel_np[0, 0:64] = 1.0
    bsel_np[32, 64:128] = 1.0
    for core in range(n_cores):
        b = core // (n_cores // b_)
        h0 = (core % (n_cores // b_)) * hpc
        heads = list(range(h0, h0 + hpc))
        cols = np.concatenate([np.arange(h * d_, (h + 1) * d_) for h in heads])
        x_t = np.ascontiguousarray(x[b].T)                       # [C, T]
        w_qk = np.ascontiguousarray(
            np.concatenate([Wq[:, cols], Wk[:, cols]], axis=1))  # [C, 512]
        w_v = np.ascontiguousarray(Wv[:, cols])                  # [C, 256]
        npair = hpc // 2
        ckt_np = np.zeros((npair, 128, p_), np.float32)
        cv_np = np.zeros((npair, 128, p_ // KC, 128), np.float32)
        wp_np = np.zeros((npair, 128, c_), np.float32)
        for pr in range(npair):
            he, ho = heads[2 * pr], heads[2 * pr + 1]
            ckt_np[pr, 0:64] = cache_k[b, he].T
            ckt_np[pr, 64:128] = cache_k[b, ho].T
            cvr_e = cache_v[b, he].reshape(p_ // KC, KC, d_)     # [chunk, key, d]
            cvr_o = cache_v[b, ho].reshape(p_ // KC, KC, d_)
            cv_np[pr, :, :, 0:64] = cvr_e.transpose(1, 0, 2)
            cv_np[pr, :, :, 64:128] = cvr_o.transpose(1, 0, 2)
            wp_np[pr, 0:64] = W_proj[he * d_:(he + 1) * d_]
            wp_np[pr, 64:128] = W_proj[ho * d_:(ho + 1) * d_]
        in_maps.append({
            "x_t": x_t,
            "w_qk": w_qk,
            "w_v": w_v,
            "ckt": ckt_np,
            "cv": cv_np.astype(ml_dtypes.bfloat16),
            "wp": wp_np,
            "masks": mask_np.astype(ml_dtypes.bfloat16),
            "ones": ones_np.astype(ml_dtypes.bfloat16),
            "bsel": bsel_np,
            "zrd": np.zeros((33, TQ), np.float32),
        })
    return in_maps


def assemble_output(results, n_cores=8, b_=B, t_=T, c_=C):
    """Sum per-core partial out^T over head groups, transpose back."""
    out = np.zeros((b_, t_, c_), np.float32)
    per_b = n_cores // b_
    for b in range(b_):
        acc = np.zeros((c_, t_), np.float32)
        for i in range(per_b):
            acc += results[b * per_b + i]["out_t"]
        out[b] = acc.T
    return out


_NC_CACHE = {}


def kernel(x, W_attn, W_proj, cache_k, cache_v):
    x = np.asarray(x, np.float32)
    W_attn = np.asarray(W_attn, np.float32)
    W_proj = np.asarray(W_proj, np.float32)
    cache_k = np.asarray(cache_k, np.float32)
    cache_v = np.asarray(cache_v, np.float32)
    if "nc" not in _NC_CACHE:
        _NC_CACHE["nc"] = build_kernel()
    nc = _NC_CACHE["nc"]
    in_maps = make_in_maps(x, W_attn, W_proj, cache_k, cache_v)
    res = run_bass_kernel_spmd(nc, in_maps, list(range(8)))
    return assemble_output(res.results)



# revision 13
# speedup vs baseline: 1.7902x; 1.7902x over previous
"""Trainium2 Bass kernel for CausalSelfAttention with KV-prefix cache (v2).

Problem (hardcoded): B=2, T=2048, C=1024, H=16, D=64, P=2048.
Sharding: 8 cores = 2 (batch) x 4 (head groups of 4 heads).

v2 design vs v1 baseline (698us):
 - denominator rows come free from the AV matmuls via a ones column
   appended to V (M=65, per-head PSUM banks); the per-chunk [1,512]
   denominator matmuls and their LDWEIGHTS are gone (-110us PE busy).
 - scores PSUM is double-buffered so the PE streams scores for chunk
   g+1 while ScalarE exps chunk g (v1's single 6-bank group serialized
   PE against ScalarE for ~180us of idle).
 - everything is bf16 (x, W, q/k, cache_k, y, W_proj, output partials);
   fp8 was measured to blow the 2e-2 rel-err budget (2.4e-2 on CPU sim).
 - ScalarE runs Exp exclusively; every PSUM->SBUF copy is on DVE.
 - remaining QKV/V matmul blocks are woven into the attention loop with
   static deadlines so phase 1 hides in ScalarE's shadow.
 - y+denominator PSUM is copied to SBUF right after the last AV so the
   single-buffered y banks free immediately; reciprocal/broadcast/
   normalize run from SBUF off the critical path.
"""

import numpy as np
import ml_dtypes
from contextlib import ExitStack

import concourse.bacc as bacc
import concourse.tile as tile
import concourse.mybir as mybir
from concourse.bass_utils import run_bass_kernel_spmd

F32 = mybir.dt.float32
F32R = mybir.dt.float32r
BF16 = mybir.dt.bfloat16
EXP = mybir.ActivationFunctionType.Exp
MULT = mybir.AluOpType.mult

B, T, C, H, D, P = 2, 2048, 1024, 16, 64, 2048
HPC = 4            # heads per core
NPAIR = 2          # head pairs per core
TQ = 512           # query block (matmul moving dim)
KC = 128           # key chunk (PSUM partition dim)
SCALE = 1.0 / np.sqrt(D)

NT = T // TQ       # 4  query blocks
NPC = P // KC      # 16 prefix key chunks
NCK = C // 128     # 8  C contraction chunks
NTC = T // 128     # 16 current-key 128-chunks


def build_kernel(n_cores=8, dbg=False):
    nc = bacc.Bacc("TRN2", target_bir_lowering=False, debug=False,
                   num_devices=n_cores)
    dbg_t = {}
    if dbg:
        for nm, shp, dt_ in [("dbg_qT", [128, T], BF16), ("dbg_kT", [128, T], BF16),
                             ("dbg_vt", [128, NTC, HPC, 66], BF16),
                             ("dbg_eb", [128, 2, TQ], BF16),
                             ("dbg_yb", [65, TQ], F32),
                             ("dbg_ysb", [128, NT, TQ], BF16)]:
            dbg_t[nm] = nc.dram_tensor(nm, shp, dt_, kind="ExternalOutput").ap()

    xt = nc.dram_tensor("xt", [128, NCK, T], BF16, kind="ExternalInput").ap()
    wqk = nc.dram_tensor("wqk", [128, NCK, 4 * 128], BF16, kind="ExternalInput").ap()
    wv = nc.dram_tensor("wv", [128, NCK, HPC * D], BF16, kind="ExternalInput").ap()
    ckt = nc.dram_tensor("ckt", [NPAIR, 128, P], BF16, kind="ExternalInput").ap()
    cv = nc.dram_tensor("cv", [NPAIR, 128, 2, NPC, 66], BF16, kind="ExternalInput").ap()
    wp = nc.dram_tensor("wp", [NPAIR, 128, C], BF16, kind="ExternalInput").ap()
    masks = nc.dram_tensor("masks", [128, 4, TQ], BF16, kind="ExternalInput").ap()
    bsel = nc.dram_tensor("bsel", [65, 64], F32, kind="ExternalInput").ap()
    vones = nc.dram_tensor("vones", [128, NTC, HPC, 2], BF16, kind="ExternalInput").ap()
    zrd = nc.dram_tensor("zrd", [65, TQ], F32, kind="ExternalInput").ap()
    out_t = nc.dram_tensor("out_t", [C, T], BF16, kind="ExternalOutput").ap()

    with tile.TileContext(nc) as tc, ExitStack() as top:
        const = top.enter_context(tc.tile_pool(name="const", bufs=1))
        persist = top.enter_context(tc.tile_pool(name="persist", bufs=1))

        # ---- persistent SBUF ---------------------------------------------
        qT = [persist.tile([128, T], BF16, tag=f"qT{i}", name=f"qT{i}") for i in range(NPAIR)]
        kT = [persist.tile([128, T], BF16, tag=f"kT{i}", name=f"kT{i}") for i in range(NPAIR)]
        cktT = [persist.tile([128, P], BF16, tag=f"cktT{i}", name=f"cktT{i}") for i in range(NPAIR)]
        cvt = [persist.tile([128, 2, NPC, 66], BF16, tag=f"cvt{i}", name=f"cvt{i}") for i in range(NPAIR)]
        vt = persist.tile([128, NTC, HPC, 66], BF16, tag="vt", name="vt")
        wpt = [persist.tile([128, C], BF16, tag=f"wpt{i}", name=f"wpt{i}") for i in range(NPAIR)]
        ysb = [persist.tile([128, NT, TQ], BF16, tag=f"ysb{i}", name=f"ysb{i}") for i in range(NPAIR)]
        rdE = persist.tile([65, TQ], F32R, tag="rdE", name="rdE")
        rdO = persist.tile([65, TQ], F32R, tag="rdO", name="rdO")
        maskt = const.tile([128, 4, TQ], BF16, tag="maskt", name="maskt")
        bselt = const.tile([65, 64], F32R, tag="bselt", name="bselt")
        xtT = persist.tile([128, NCK, T], BF16, tag="xtT", name="xtT")
        wqkT = persist.tile([128, NCK, 4 * 128], BF16, tag="wqkT", name="wqkT")
        wvT = persist.tile([128, NCK, HPC * D], BF16, tag="wvT", name="wvT")

        # ---- input DMAs (ordered so early consumers land first) ---------
        nc.sync.dma_start(maskt[:], masks[:, :, :])
        nc.sync.dma_start(bselt[:], bsel[:, :].bitcast(F32R))
        for kc_ in range(NCK):
            nc.sync.dma_start(wqkT[:, kc_, :], wqk[:, kc_, :])
        for kc_ in range(NCK):
            nc.sync.dma_start(xtT[:, kc_, 0:TQ], xt[:, kc_, 0:TQ])
        for pr in range(NPAIR):
            for j in range(4):
                nc.sync.dma_start(cktT[pr][:, j * TQ:(j + 1) * TQ],
                                  ckt[pr, :, j * TQ:(j + 1) * TQ])
        for nb in range(1, NT):
            for kc_ in range(NCK):
                nc.sync.dma_start(xtT[:, kc_, nb * TQ:(nb + 1) * TQ],
                                  xt[:, kc_, nb * TQ:(nb + 1) * TQ])
        for kc_ in range(NCK):
            nc.sync.dma_start(wvT[:, kc_, :], wv[:, kc_, :])
        for pr in range(NPAIR):
            nc.sync.dma_start(cvt[pr][:], cv[pr, :, :, :, :])
            nc.sync.dma_start(wpt[pr][:], wp[pr, :, :])

        nc.sync.dma_start(vt[:, :, :, 64:66], vones[:, :, :, :])
        nc.sync.dma_start(rdE[:], zrd[:, :].bitcast(F32R))
        nc.sync.dma_start(rdO[:], zrd[:, :].bitcast(F32R))

        with ExitStack() as main:
            psA = main.enter_context(tc.tile_pool(name="psA", bufs=2, space="PSUM"))
            psY = main.enter_context(tc.tile_pool(name="psY", bufs=1, space="PSUM"))
            ps1 = main.enter_context(tc.tile_pool(name="ps1", bufs=2, space="PSUM"))
            ebp = main.enter_context(tc.tile_pool(name="ebp", bufs=3))
            ybf = main.enter_context(tc.tile_pool(name="ybf", bufs=2))
            ytmp = main.enter_context(tc.tile_pool(name="ytmp", bufs=2))

            # ---- phase-1 block thunks ------------------------------------
            # mc: 0 = q pair0, 1 = q pair1, 2 = k pair0, 3 = k pair1
            def qk_block_thunks(mc, nb):
                dest = (qT[0], qT[1], kT[0], kT[1])[mc]
                cell = {}
                thunks = []

                def mk_mm(kc_):
                    def f():
                        if "ps" not in cell:
                            cell["ps"] = ps1.tile([128, TQ], F32, tag="p1", name="p1")
                        nc.tensor.matmul(
                            cell["ps"][:],
                            wqkT[:, kc_, mc * 128:(mc + 1) * 128],
                            xtT[:, kc_, nb * TQ:(nb + 1) * TQ],
                            start=(kc_ == 0), stop=(kc_ == NCK - 1),
                            skip_group_check=True)
                    return f

                for kc_ in range(NCK):
                    thunks.append(mk_mm(kc_))

                def fin():
                    with nc.allow_low_precision(reason="q/k psum -> bf16 SBUF"):
                        nc.vector.tensor_copy(
                            dest[:, nb * TQ:(nb + 1) * TQ], cell["ps"][:])
                thunks.append(fin)
                return thunks

            def v_block_thunks(tc_):
                cell = {}
                thunks = []

                def mk_mm(kc_):
                    def f():
                        if "ps" not in cell:
                            cell["ps"] = ps1.tile([128, TQ], F32, tag="p1", name="p1")
                        nc.tensor.matmul(
                            cell["ps"][:, 0:HPC * D],
                            xtT[:, kc_, tc_ * 128:(tc_ + 1) * 128],
                            wvT[:, kc_, :],
                            start=(kc_ == 0), stop=(kc_ == NCK - 1),
                            skip_group_check=True)
                    return f

                for kc_ in range(NCK):
                    thunks.append(mk_mm(kc_))

                def fin():
                    with nc.allow_low_precision(reason="v psum -> bf16 SBUF"):
                        nc.vector.tensor_copy(
                            vt[:, tc_, :, 0:64], cell["ps"][:, 0:HPC * D])
                thunks.append(fin)
                return thunks

            # upfront: everything tb0 needs (q/k nb0, v chunks 0-3)
            for mc in (0, 2, 1, 3):
                for th in qk_block_thunks(mc, 0):
                    th()
            for tc_ in range(4):
                for th in v_block_thunks(tc_):
                    th()

            # ---- weave schedule for the remaining blocks -----------------
            tb_start = {}
            gc = 0
            for tb in range(NT):
                for pr in range(NPAIR):
                    tb_start[(tb, pr)] = gc
                    gc += NPC + 4 * (tb + 1)
            total_chunks = gc  # 208

            # blocks emit atomically (all 9 instructions at one slot) so the
            # ps1 ring never holds a partially-emitted accumulation when a
            # later alloc (bcp) wants the slot back.
            blocks = []
            for nb in range(1, NT):
                for mc in (0, 1):   # q blocks: needed at tb=nb start
                    blocks.append((tb_start[(nb, 0)], qk_block_thunks(mc, nb)))
                for mc in (2, 3):   # k block nb first used at current chunk 4*nb
                    blocks.append((tb_start[(nb, 0)] + NPC + 4 * nb,
                                   qk_block_thunks(mc, nb)))
            for tc_ in range(4, NTC):
                tb = tc_ // 4      # vt[tc] first used at current chunk tc of tb
                blocks.append((tb_start[(tb, 0)] + NPC + tc_, v_block_thunks(tc_)))
            blocks.sort(key=lambda x: x[0])

            schedule = {}
            prev = 20
            for dl, ths in blocks:
                slot = min(max(prev + 5, dl - 30), dl - 1)
                assert 20 <= slot < dl <= total_chunks, (slot, dl)
                prev = slot
                schedule.setdefault(slot, []).extend(ths)

            # ---- main attention loop -------------------------------------
            gc = 0
            for tb in range(NT):
                for pr in range(NPAIR):
                    nkc = NPC + 4 * (tb + 1)
                    ybe = psY.tile([65, TQ], F32, tag="yE", name="yE")
                    ybo = psY.tile([65, TQ], F32, tag="yO", name="yO")
                    for c in range(nkc):
                        for th in schedule.pop(gc, ()):
                            th()
                        gc += 1
                        if c < NPC:
                            ksrc, klo = cktT[pr], c * KC
                        else:
                            ksrc, klo = kT[pr], (c - NPC) * KC
                        sb = psA.tile([128, 2, TQ], F32, tag="sb", name="sb")
                        for h in range(2):
                            nc.tensor.matmul(
                                sb[:, h, :],
                                ksrc[h * 64:(h + 1) * 64, klo:klo + KC],
                                qT[pr][h * 64:(h + 1) * 64, tb * TQ:(tb + 1) * TQ],
                                start=True, stop=True,
                                tile_position=(h * 64, 0),
                                skip_group_check=True)
                        eb = ebp.tile([128, 2, TQ], BF16, tag="eb", name="eb")
                        nc.scalar.activation(eb[:], sb[:], EXP, scale=SCALE)
                        if c >= NPC:
                            j = (c - NPC) - 4 * tb
                            if j >= 0:
                                for h in range(2):
                                    nc.vector.tensor_tensor(
                                        eb[:, h, :], eb[:, h, :],
                                        maskt[:, j, :], MULT)
                        if dbg and tb == 0 and pr == 0 and c == 0:
                            nc.sync.dma_start(dbg_t["dbg_eb"][:, :, :], eb[:])
                        if c < NPC:
                            vsrc_e = cvt[pr][:, 0, c, 0:65]
                            vsrc_o = cvt[pr][:, 1, c, 0:65]
                        else:
                            ck = c - NPC
                            vsrc_e = vt[:, ck, 2 * pr + 0, 0:65]
                            vsrc_o = vt[:, ck, 2 * pr + 1, 0:65]
                        st, sp = (c == 0), (c == nkc - 1)
                        nc.tensor.matmul(ybe[:], vsrc_e, eb[:, 0, :],
                                         start=st, stop=sp,
                                         tile_position=(0, 0),
                                         skip_group_check=True)
                        nc.tensor.matmul(ybo[:], vsrc_o, eb[:, 1, :],
                                         start=st, stop=sp,
                                         tile_position=(0, 0),
                                         skip_group_check=True)

                    # ---- tail: free y banks fast, then normalize ---------
                    ybfE = ybf.tile([65, TQ], F32, tag="ybf", name="ybfE")
                    ybfO = ybf.tile([65, TQ], F32, tag="ybf", name="ybfO")
                    nc.vector.tensor_copy(ybfE[:], ybe[:])
                    nc.vector.tensor_copy(ybfO[:], ybo[:])
                    with nc.allow_low_precision(reason="recip -> f32r for bcast mm"):
                        nc.vector.reciprocal(rdE[64:65, :], ybfE[64:65, :])
                        nc.vector.reciprocal(rdO[64:65, :], ybfO[64:65, :])
                    bcpE = ps1.tile([128, TQ], F32, tag="p1", name="bcpE")
                    bcpO = ps1.tile([128, TQ], F32, tag="p1", name="bcpO")
                    nc.tensor.matmul(bcpE[0:64, :], bselt[:], rdE[:],
                                     start=True, stop=True,
                                     skip_group_check=True)
                    nc.tensor.matmul(bcpO[0:64, :], bselt[:], rdO[:],
                                     start=True, stop=True,
                                     skip_group_check=True)
                    with nc.allow_low_precision(reason="normalize -> bf16 y"):
                        nc.vector.tensor_tensor(
                            ysb[pr][0:64, tb, :], ybfE[0:64, :],
                            bcpE[0:64, :], MULT)
                        yo = ytmp.tile([64, TQ], BF16, tag="yo", name="yo")
                        nc.vector.tensor_tensor(
                            yo[:], ybfO[0:64, :], bcpO[0:64, :], MULT)
                    nc.sync.dma_start(ysb[pr][64:128, tb, :], yo[:])
                    if dbg and tb == 0 and pr == 0:
                        nc.sync.dma_start(dbg_t["dbg_yb"][:, :], ybfE[:])

            assert not schedule, f"unemitted extras: {sorted(schedule)}"
            if dbg:
                nc.sync.dma_start(dbg_t["dbg_qT"][:, :], qT[0][:])
                nc.sync.dma_start(dbg_t["dbg_kT"][:, :], kT[0][:])
                nc.sync.dma_start(dbg_t["dbg_vt"][:, :, :, :], vt[:])
                nc.sync.dma_start(dbg_t["dbg_ysb"][:, :, :], ysb[0][:])

        # ---- output projection ------------------------------------------
        with ExitStack() as ph3:
            pso = ph3.enter_context(tc.tile_pool(name="pso", bufs=4, space="PSUM"))
            stg = ph3.enter_context(tc.tile_pool(name="stg", bufs=4))
            for nb in range(NT):
                for mc in range(C // 128):
                    ps = pso.tile([128, TQ], F32, tag="pso", name="pso")
                    for pr in range(NPAIR):
                        nc.tensor.matmul(
                            ps[:],
                            wpt[pr][:, mc * 128:(mc + 1) * 128],
                            ysb[pr][:, nb, :],
                            start=(pr == 0), stop=(pr == NPAIR - 1),
                            skip_group_check=True)
                    ot = stg.tile([128, TQ], BF16, tag="ot", name="ot")
                    with nc.allow_low_precision(reason="out psum -> bf16"):
                        nc.vector.tensor_copy(ot[:], ps[:])
                    nc.sync.dma_start(
                        out_t[mc * 128:(mc + 1) * 128, nb * TQ:(nb + 1) * TQ],
                        ot[:])

    nc.compile()
    return nc


def make_in_maps(x, W_attn, W_proj, cache_k, cache_v, n_cores=8):
    """Shard full inputs into per-core input maps (host side)."""
    b_, t_, c_ = x.shape
    h_ = cache_k.shape[1]
    d_ = c_ // h_
    p_ = cache_k.shape[2]
    hpc = h_ // (n_cores // b_)
    Wq = W_attn[:, 0 * c_:1 * c_]
    Wk = W_attn[:, 1 * c_:2 * c_]
    Wv = W_attn[:, 2 * c_:3 * c_]
    mask_np = np.zeros((128, 4, TQ), np.float32)
    for j in range(4):
        mask_np[:, j, :] = (np.arange(TQ)[None, :] >=
                            (np.arange(128)[:, None] + j * 128)).astype(np.float32)
    bsel_np = np.zeros((65, 64), np.float32)
    bsel_np[64, :] = 1.0
    in_maps = []
    for core in range(n_cores):
        b = core // (n_cores // b_)
        h0 = (core % (n_cores // b_)) * hpc
        heads = list(range(h0, h0 + hpc))
        cols = np.concatenate([np.arange(h * d_, (h + 1) * d_) for h in heads])
        # x^T chunked: xt[p, kc, t] = x[b, t, kc*128+p]
        xt_np = np.ascontiguousarray(
            x[b].T.reshape(NCK, 128, t_).transpose(1, 0, 2))
        # W cols: [q pair0 | q pair1 | k pair0 | k pair1], each 128 wide
        wqk_cols = np.concatenate(
            [Wq[:, cols[0:128]], Wq[:, cols[128:256]],
             Wk[:, cols[0:128]], Wk[:, cols[128:256]]], axis=1)
        wqk_np = np.ascontiguousarray(
            wqk_cols.reshape(NCK, 128, 512).transpose(1, 0, 2))
        wv_np = np.ascontiguousarray(
            Wv[:, cols].reshape(NCK, 128, 256).transpose(1, 0, 2))
        npair = hpc // 2
        ckt_np = np.zeros((npair, 128, p_), np.float32)
        cv_np = np.zeros((npair, 128, 2, NPC, 66), np.float32)
        wp_np = np.zeros((npair, 128, c_), np.float32)
        for pr in range(npair):
            he, ho = heads[2 * pr], heads[2 * pr + 1]
            ckt_np[pr, 0:64] = cache_k[b, he].T
            ckt_np[pr, 64:128] = cache_k[b, ho].T
            for hh, hd in ((0, he), (1, ho)):
                cvr = cache_v[b, hd].reshape(NPC, KC, d_)   # [chunk, key, d]
                cv_np[pr, :, hh, :, 0:64] = cvr.transpose(1, 0, 2)
                cv_np[pr, :, hh, :, 64] = 1.0
            wp_np[pr, 0:64] = W_proj[he * d_:(he + 1) * d_]
            wp_np[pr, 64:128] = W_proj[ho * d_:(ho + 1) * d_]
        in_maps.append({
            "xt": xt_np.astype(ml_dtypes.bfloat16),
            "wqk": wqk_np.astype(ml_dtypes.bfloat16),
            "wv": wv_np.astype(ml_dtypes.bfloat16),
            "ckt": ckt_np.astype(ml_dtypes.bfloat16),
            "cv": cv_np.astype(ml_dtypes.bfloat16),
            "wp": wp_np.astype(ml_dtypes.bfloat16),
            "masks": mask_np.astype(ml_dtypes.bfloat16),
            "bsel": bsel_np,
            "vones": np.ones((128, NTC, HPC, 2), ml_dtypes.bfloat16),
            "zrd": np.zeros((65, TQ), np.float32),
        })
    return in_maps


def assemble_output(results, n_cores=8, b_=B, t_=T, c_=C):
    """Sum per-core partial out^T over head groups, transpose back."""
    out = np.zeros((b_, t_, c_), np.float32)
    per_b = n_cores // b_
    for b in range(b_):
        acc = np.zeros((c_, t_), np.float32)
        for i in range(per_b):
            acc += results[b * per_b + i]["out_t"].astype(np.float32)
        out[b] = acc.T
    return out


_NC_CACHE = {}


def kernel(x, W_attn, W_proj, cache_k, cache_v):
    x = np.asarray(x, np.float32)
    W_attn = np.asarray(W_attn, np.float32)
    W_proj = np.asarray(W_proj, np.float32)
    cache_k = np.asarray(cache_k, np.float32)
    cache_v = np.asarray(cache_v, np.float32)
    if "nc" not in _NC_CACHE:
        _NC_CACHE["nc"] = build_kernel()
    nc = _NC_CACHE["nc"]
    in_maps = make_in_maps(x, W_attn, W_proj, cache_k, cache_v)
    res = run_bass_kernel_spmd(nc, in_maps, list(range(8)))
    return assemble_output(res.results)


# revision 21
# speedup vs baseline: 1.9217x; 1.0735x over previous
"""Trainium2 Bass kernel for CausalSelfAttention with KV-prefix cache (v2).

Problem (hardcoded): B=2, T=2048, C=1024, H=16, D=64, P=2048.
Sharding: 8 cores = 2 (batch) x 4 (head groups of 4 heads).

v2 design vs v1 baseline (698us):
 - denominator rows come free from the AV matmuls via a ones column
   appended to V (M=65, per-head PSUM banks); the per-chunk [1,512]
   denominator matmuls and their LDWEIGHTS are gone (-110us PE busy).
 - scores PSUM is double-buffered so the PE streams scores for chunk
   g+1 while ScalarE exps chunk g (v1's single 6-bank group serialized
   PE against ScalarE for ~180us of idle).
 - everything is bf16 (x, W, q/k, cache_k, y, W_proj, output partials);
   fp8 was measured to blow the 2e-2 rel-err budget (2.4e-2 on CPU sim).
 - ScalarE runs Exp exclusively; every PSUM->SBUF copy is on DVE.
 - remaining QKV/V matmul blocks are woven into the attention loop with
   static deadlines so phase 1 hides in ScalarE's shadow.
 - y+denominator PSUM is copied to SBUF right after the last AV so the
   single-buffered y banks free immediately; reciprocal/broadcast/
   normalize run from SBUF off the critical path.
"""

import numpy as np
import ml_dtypes
from contextlib import ExitStack

import concourse.bacc as bacc
import concourse.tile as tile
import concourse.mybir as mybir
from concourse.bass_utils import run_bass_kernel_spmd

F32 = mybir.dt.float32
F32R = mybir.dt.float32r
BF16 = mybir.dt.bfloat16
EXP = mybir.ActivationFunctionType.Exp
MULT = mybir.AluOpType.mult

B, T, C, H, D, P = 2, 2048, 1024, 16, 64, 2048
HPC = 4            # heads per core
NPAIR = 2          # head pairs per core
TQ = 512           # query block (matmul moving dim)
KC = 128           # key chunk (PSUM partition dim)
SCALE = 1.0 / np.sqrt(D)

NT = T // TQ       # 4  query blocks
NPC = P // KC      # 16 prefix key chunks
NCK = C // 128     # 8  C contraction chunks
NTC = T // 128     # 16 current-key 128-chunks


def build_kernel(n_cores=8, dbg=False):
    nc = bacc.Bacc("TRN2", target_bir_lowering=False, debug=False,
                   num_devices=n_cores)
    dbg_t = {}
    if dbg:
        for nm, shp, dt_ in [("dbg_qT", [128, T], BF16), ("dbg_kT", [128, T], BF16),
                             ("dbg_vt", [128, NTC, HPC, 66], BF16),
                             ("dbg_eb", [128, 2, TQ], BF16),
                             ("dbg_yb", [65, TQ], F32),
                             ("dbg_ysb", [128, NT, TQ], BF16)]:
            dbg_t[nm] = nc.dram_tensor(nm, shp, dt_, kind="ExternalOutput").ap()

    xt = nc.dram_tensor("xt", [128, NCK, T], BF16, kind="ExternalInput").ap()
    wqk = nc.dram_tensor("wqk", [128, NCK, 4 * 128], BF16, kind="ExternalInput").ap()
    wv = nc.dram_tensor("wv", [128, NCK, HPC * D], BF16, kind="ExternalInput").ap()
    ckt = nc.dram_tensor("ckt", [NPAIR, 128, P], BF16, kind="ExternalInput").ap()
    cv = nc.dram_tensor("cv", [NPAIR, 128, 2, NPC, 66], BF16, kind="ExternalInput").ap()
    wp = nc.dram_tensor("wp", [NPAIR, 128, C], BF16, kind="ExternalInput").ap()
    masks = nc.dram_tensor("masks", [128, 4, TQ], BF16, kind="ExternalInput").ap()
    bsel = nc.dram_tensor("bsel", [65, 64], F32, kind="ExternalInput").ap()
    vones = nc.dram_tensor("vones", [128, NTC, HPC, 2], BF16, kind="ExternalInput").ap()
    zrd = nc.dram_tensor("zrd", [65, TQ], F32, kind="ExternalInput").ap()
    out_t = nc.dram_tensor("out_t", [C, T], BF16, kind="ExternalOutput").ap()

    with tile.TileContext(nc) as tc, ExitStack() as top:
        const = top.enter_context(tc.tile_pool(name="const", bufs=1))
        persist = top.enter_context(tc.tile_pool(name="persist", bufs=1))

        # ---- persistent SBUF ---------------------------------------------
        qT = [persist.tile([128, T], BF16, tag=f"qT{i}", name=f"qT{i}") for i in range(NPAIR)]
        kT = [persist.tile([128, T], BF16, tag=f"kT{i}", name=f"kT{i}") for i in range(NPAIR)]
        cktT = [persist.tile([128, P], BF16, tag=f"cktT{i}", name=f"cktT{i}") for i in range(NPAIR)]
        cvt = [persist.tile([128, 2, NPC, 66], BF16, tag=f"cvt{i}", name=f"cvt{i}") for i in range(NPAIR)]
        vt = persist.tile([128, NTC, HPC, 66], BF16, tag="vt", name="vt")
        wpt = [persist.tile([128, C], BF16, tag=f"wpt{i}", name=f"wpt{i}") for i in range(NPAIR)]
        ysb = [persist.tile([128, NT, TQ], BF16, tag=f"ysb{i}", name=f"ysb{i}") for i in range(NPAIR)]
        rdE = persist.tile([65, TQ], F32R, tag="rdE", name="rdE")
        rdO = persist.tile([65, TQ], F32R, tag="rdO", name="rdO")
        maskt = const.tile([128, 4, TQ], BF16, tag="maskt", name="maskt")
        bselt = const.tile([65, 64], F32R, tag="bselt", name="bselt")
        xtT = persist.tile([128, NCK, T], BF16, tag="xtT", name="xtT")
        wqkT = persist.tile([128, NCK, 4 * 128], BF16, tag="wqkT", name="wqkT")
        wvT = persist.tile([128, NCK, HPC * D], BF16, tag="wvT", name="wvT")

        # ---- input DMAs: the first 16 land on all 16 queues, so the
        # pieces the first matmuls/scores need go first.
        for kc_ in range(NCK):
            nc.sync.dma_start(xtT[:, kc_, 0:TQ], xt[:, kc_, 0:TQ])
        for kc_ in range(NCK):
            nc.sync.dma_start(wqkT[:, kc_, :], wqk[:, kc_, :])
        for j in range(4):
            nc.sync.dma_start(cktT[0][:, j * TQ:(j + 1) * TQ],
                              ckt[0, :, j * TQ:(j + 1) * TQ])
        for kc_ in range(NCK):
            nc.sync.dma_start(wvT[:, kc_, :], wv[:, kc_, :])
        nc.sync.dma_start(bselt[:], bsel[:, :].bitcast(F32R))
        for j in range(4):
            nc.sync.dma_start(maskt[:, j, :], masks[:, j, :])
        for j in range(4):
            nc.sync.dma_start(cktT[1][:, j * TQ:(j + 1) * TQ],
                              ckt[1, :, j * TQ:(j + 1) * TQ])
        nc.sync.dma_start(cvt[0][:], cv[0, :, :, :, :])
        for nb in range(1, NT):
            for kc_ in range(NCK):
                nc.sync.dma_start(xtT[:, kc_, nb * TQ:(nb + 1) * TQ],
                                  xt[:, kc_, nb * TQ:(nb + 1) * TQ])
        nc.sync.dma_start(cvt[1][:], cv[1, :, :, :, :])
        for pr in range(NPAIR):
            nc.sync.dma_start(wpt[pr][:], wp[pr, :, :])
        nc.sync.dma_start(vt[:, :, :, 64:66], vones[:, :, :, :])
        nc.sync.dma_start(rdE[:], zrd[:, :].bitcast(F32R))
        nc.sync.dma_start(rdO[:], zrd[:, :].bitcast(F32R))

        with ExitStack() as main:
            psA = main.enter_context(tc.tile_pool(name="psA", bufs=2, space="PSUM"))
            psY = main.enter_context(tc.tile_pool(name="psY", bufs=1, space="PSUM"))
            ps1 = main.enter_context(tc.tile_pool(name="ps1", bufs=2, space="PSUM"))
            ebp = main.enter_context(tc.tile_pool(name="ebp", bufs=3))
            ybf = main.enter_context(tc.tile_pool(name="ybf", bufs=4))
            ytmp = main.enter_context(tc.tile_pool(name="ytmp", bufs=2))

            # ---- phase-1 block thunks ------------------------------------
            # mc: 0 = q pair0, 1 = q pair1, 2 = k pair0, 3 = k pair1
            def qk_block_thunks(mc, nb):
                dest = (qT[0], qT[1], kT[0], kT[1])[mc]
                cell = {}
                thunks = []

                def mk_mm(kc_):
                    def f():
                        if "ps" not in cell:
                            cell["ps"] = ps1.tile([128, TQ], F32, tag="p1", name="p1")
                        nc.tensor.matmul(
                            cell["ps"][:],
                            wqkT[:, kc_, mc * 128:(mc + 1) * 128],
                            xtT[:, kc_, nb * TQ:(nb + 1) * TQ],
                            start=(kc_ == 0), stop=(kc_ == NCK - 1),
                            skip_group_check=True)
                    return f

                for kc_ in range(NCK):
                    thunks.append(mk_mm(kc_))

                def fin():
                    with nc.allow_low_precision(reason="q/k psum -> bf16 SBUF"):
                        nc.vector.tensor_copy(
                            dest[:, nb * TQ:(nb + 1) * TQ], cell["ps"][:])
                thunks.append(fin)
                return thunks

            def v_block_thunks(tc_):
                cell = {}
                thunks = []

                def mk_mm(kc_):
                    def f():
                        if "ps" not in cell:
                            cell["ps"] = ps1.tile([128, TQ], F32, tag="p1", name="p1")
                        nc.tensor.matmul(
                            cell["ps"][:, 0:HPC * D],
                            xtT[:, kc_, tc_ * 128:(tc_ + 1) * 128],
                            wvT[:, kc_, :],
                            start=(kc_ == 0), stop=(kc_ == NCK - 1),
                            skip_group_check=True)
                    return f

                for kc_ in range(NCK):
                    thunks.append(mk_mm(kc_))

                def fin():
                    with nc.allow_low_precision(reason="v psum -> bf16 SBUF"):
                        nc.vector.tensor_copy(
                            vt[:, tc_, :, 0:64], cell["ps"][:, 0:HPC * D])
                thunks.append(fin)
                return thunks

            # upfront: everything tb0 needs (q/k nb0, v chunks 0-3)
            for mc in (0, 2, 1, 3):
                for th in qk_block_thunks(mc, 0):
                    th()
            for tc_ in range(4):
                for th in v_block_thunks(tc_):
                    th()

            # ---- weave schedule for the remaining blocks -----------------
            tb_start = {}
            gc = 0
            for tb in range(NT):
                for pr in range(NPAIR):
                    tb_start[(tb, pr)] = gc
                    gc += NPC + 4 * (tb + 1)
            total_chunks = gc  # 208

            # blocks emit atomically (all 9 instructions at one slot) so the
            # ps1 ring never holds a partially-emitted accumulation when a
            # later alloc (bcp) wants the slot back.
            blocks = []
            for nb in range(1, NT):
                for mc in (0, 1):   # q blocks: needed at tb=nb start
                    blocks.append((tb_start[(nb, 0)], qk_block_thunks(mc, nb)))
                for mc in (2, 3):   # k block nb first used at current chunk 4*nb
                    blocks.append((tb_start[(nb, 0)] + NPC + 4 * nb,
                                   qk_block_thunks(mc, nb)))
            for tc_ in range(4, NTC):
                tb = tc_ // 4      # vt[tc] first used at current chunk tc of tb
                blocks.append((tb_start[(tb, 0)] + NPC + tc_, v_block_thunks(tc_)))
            blocks.sort(key=lambda x: x[0])

            schedule = {}
            prev = 20
            for dl, ths in blocks:
                slot = min(max(prev + 5, dl - 30), dl - 1)
                assert 20 <= slot < dl <= total_chunks, (slot, dl)
                prev = slot
                schedule.setdefault(slot, []).extend(ths)

            # ---- main attention loop -------------------------------------
            # Tail work that would stall the PE in program order (bcast
            # matmuls waiting on the 3.3us DVE reciprocals) is deferred
            # into the next pair's chunk stream.
            pending_tail = []
            gc = 0
            for tb in range(NT):
                for pr in range(NPAIR):
                    nkc = NPC + 4 * (tb + 1)
                    ybe = psY.tile([65, TQ], F32, tag="yE", name="yE")
                    ybo = psY.tile([65, TQ], F32, tag="yO", name="yO")
                    for c in range(nkc):
                        for th in schedule.pop(gc, ()):
                            th()
                        gc += 1
                        if c == 8 and pending_tail:
                            pending_tail.pop()()
                        if c < NPC:
                            ksrc, klo = cktT[pr], c * KC
                        else:
                            ksrc, klo = kT[pr], (c - NPC) * KC
                        sb = psA.tile([128, 2, TQ], F32, tag="sb", name="sb")
                        for h in range(2):
                            nc.tensor.matmul(
                                sb[:, h, :],
                                ksrc[h * 64:(h + 1) * 64, klo:klo + KC],
                                qT[pr][h * 64:(h + 1) * 64, tb * TQ:(tb + 1) * TQ],
                                start=True, stop=True,
                                tile_position=(h * 64, 0),
                                skip_group_check=True)
                        eb = ebp.tile([128, 2, TQ], BF16, tag="eb", name="eb")
                        nc.scalar.activation(eb[:], sb[:], EXP, scale=SCALE)
                        if c >= NPC:
                            j = (c - NPC) - 4 * tb
                            if j >= 0:
                                for h in range(2):
                                    nc.vector.tensor_tensor(
                                        eb[:, h, :], eb[:, h, :],
                                        maskt[:, j, :], MULT)
                        if dbg and tb == 0 and pr == 0 and c == 0:
                            nc.sync.dma_start(dbg_t["dbg_eb"][:, :, :], eb[:])
                        if c < NPC:
                            vsrc_e = cvt[pr][:, 0, c, 0:65]
                            vsrc_o = cvt[pr][:, 1, c, 0:65]
                        else:
                            ck = c - NPC
                            vsrc_e = vt[:, ck, 2 * pr + 0, 0:65]
                            vsrc_o = vt[:, ck, 2 * pr + 1, 0:65]
                        st, sp = (c == 0), (c == nkc - 1)
                        nc.tensor.matmul(ybe[:], vsrc_e, eb[:, 0, :],
                                         start=st, stop=sp,
                                         tile_position=(0, 0),
                                         skip_group_check=True)
                        nc.tensor.matmul(ybo[:], vsrc_o, eb[:, 1, :],
                                         start=st, stop=sp,
                                         tile_position=(0, 0),
                                         skip_group_check=True)

                    # ---- tail: free y banks + start recips now; defer the
                    # PE bcast + normalize into the next pair's chunks.
                    ybfE = ybf.tile([65, TQ], F32, tag="ybf", name="ybfE")
                    ybfO = ybf.tile([65, TQ], F32, tag="ybf", name="ybfO")
                    nc.vector.tensor_copy(ybfE[:], ybe[:])
                    nc.vector.tensor_copy(ybfO[:], ybo[:])
                    with nc.allow_low_precision(reason="recip -> f32r for bcast mm"):
                        nc.vector.reciprocal(rdE[64:65, :], ybfE[64:65, :])
                        nc.vector.reciprocal(rdO[64:65, :], ybfO[64:65, :])
                    if dbg and tb == 0 and pr == 0:
                        nc.sync.dma_start(dbg_t["dbg_yb"][:, :], ybfE[:])

                    def tail(tb=tb, pr=pr, ybfE=ybfE, ybfO=ybfO):
                        bcpE = ps1.tile([128, TQ], F32, tag="p1", name="bcpE")
                        bcpO = ps1.tile([128, TQ], F32, tag="p1", name="bcpO")
                        nc.tensor.matmul(bcpE[0:64, :], bselt[:], rdE[:],
                                         start=True, stop=True,
                                         skip_group_check=True)
                        nc.tensor.matmul(bcpO[0:64, :], bselt[:], rdO[:],
                                         start=True, stop=True,
                                         skip_group_check=True)
                        with nc.allow_low_precision(reason="normalize -> bf16 y"):
                            nc.vector.tensor_tensor(
                                ysb[pr][0:64, tb, :], ybfE[0:64, :],
                                bcpE[0:64, :], MULT)
                            yo = ytmp.tile([64, TQ], BF16, tag="yo", name="yo")
                            nc.vector.tensor_tensor(
                                yo[:], ybfO[0:64, :], bcpO[0:64, :], MULT)
                        nc.sync.dma_start(ysb[pr][64:128, tb, :], yo[:])
                    pending_tail.append(tail)

            while pending_tail:
                pending_tail.pop()()
            assert not schedule, f"unemitted extras: {sorted(schedule)}"
            if dbg:
                nc.sync.dma_start(dbg_t["dbg_qT"][:, :], qT[0][:])
                nc.sync.dma_start(dbg_t["dbg_kT"][:, :], kT[0][:])
                nc.sync.dma_start(dbg_t["dbg_vt"][:, :, :, :], vt[:])
                nc.sync.dma_start(dbg_t["dbg_ysb"][:, :, :], ysb[0][:])

        # ---- output projection ------------------------------------------
        with ExitStack() as ph3:
            pso = ph3.enter_context(tc.tile_pool(name="pso", bufs=4, space="PSUM"))
            stg = ph3.enter_context(tc.tile_pool(name="stg", bufs=4))
            for nb in range(NT):
                for mc in range(C // 128):
                    ps = pso.tile([128, TQ], F32, tag="pso", name="pso")
                    for pr in range(NPAIR):
                        nc.tensor.matmul(
                            ps[:],
                            wpt[pr][:, mc * 128:(mc + 1) * 128],
                            ysb[pr][:, nb, :],
                            start=(pr == 0), stop=(pr == NPAIR - 1),
                            skip_group_check=True)
                    ot = stg.tile([128, TQ], BF16, tag="ot", name="ot")
                    with nc.allow_low_precision(reason="out psum -> bf16"):
                        nc.vector.tensor_copy(ot[:], ps[:])
                    nc.sync.dma_start(
                        out_t[mc * 128:(mc + 1) * 128, nb * TQ:(nb + 1) * TQ],
                        ot[:])

    nc.compile()
    return nc


def make_in_maps(x, W_attn, W_proj, cache_k, cache_v, n_cores=8):
    """Shard full inputs into per-core input maps (host side)."""
    b_, t_, c_ = x.shape
    h_ = cache_k.shape[1]
    d_ = c_ // h_
    p_ = cache_k.shape[2]
    hpc = h_ // (n_cores // b_)
    Wq = W_attn[:, 0 * c_:1 * c_]
    Wk = W_attn[:, 1 * c_:2 * c_]
    Wv = W_attn[:, 2 * c_:3 * c_]
    mask_np = np.zeros((128, 4, TQ), np.float32)
    for j in range(4):
        mask_np[:, j, :] = (np.arange(TQ)[None, :] >=
                            (np.arange(128)[:, None] + j * 128)).astype(np.float32)
    bsel_np = np.zeros((65, 64), np.float32)
    bsel_np[64, :] = 1.0
    in_maps = []
    for core in range(n_cores):
        b = core // (n_cores // b_)
        h0 = (core % (n_cores // b_)) * hpc
        heads = list(range(h0, h0 + hpc))
        cols = np.concatenate([np.arange(h * d_, (h + 1) * d_) for h in heads])
        # x^T chunked: xt[p, kc, t] = x[b, t, kc*128+p]
        xt_np = np.ascontiguousarray(
            x[b].T.reshape(NCK, 128, t_).transpose(1, 0, 2))
        # W cols: [q pair0 | q pair1 | k pair0 | k pair1], each 128 wide
        wqk_cols = np.concatenate(
            [Wq[:, cols[0:128]], Wq[:, cols[128:256]],
             Wk[:, cols[0:128]], Wk[:, cols[128:256]]], axis=1)
        wqk_np = np.ascontiguousarray(
            wqk_cols.reshape(NCK, 128, 512).transpose(1, 0, 2))
        wv_np = np.ascontiguousarray(
            Wv[:, cols].reshape(NCK, 128, 256).transpose(1, 0, 2))
        npair = hpc // 2
        ckt_np = np.zeros((npair, 128, p_), np.float32)
        cv_np = np.zeros((npair, 128, 2, NPC, 66), np.float32)
        wp_np = np.zeros((npair, 128, c_), np.float32)
        for pr in range(npair):
            he, ho = heads[2 * pr], heads[2 * pr + 1]
            ckt_np[pr, 0:64] = cache_k[b, he].T
            ckt_np[pr, 64:128] = cache_k[b, ho].T
            for hh, hd in ((0, he), (1, ho)):
                cvr = cache_v[b, hd].reshape(NPC, KC, d_)   # [chunk, key, d]
                cv_np[pr, :, hh, :, 0:64] = cvr.transpose(1, 0, 2)
                cv_np[pr, :, hh, :, 64] = 1.0
            wp_np[pr, 0:64] = W_proj[he * d_:(he + 1) * d_]
            wp_np[pr, 64:128] = W_proj[ho * d_:(ho + 1) * d_]
        in_maps.append({
            "xt": xt_np.astype(ml_dtypes.bfloat16),
            "wqk": wqk_np.astype(ml_dtypes.bfloat16),
            "wv": wv_np.astype(ml_dtypes.bfloat16),
            "ckt": ckt_np.astype(ml_dtypes.bfloat16),
            "cv": cv_np.astype(ml_dtypes.bfloat16),
            "wp": wp_np.astype(ml_dtypes.bfloat16),
            "masks": mask_np.astype(ml_dtypes.bfloat16),
            "bsel": bsel_np,
            "vones": np.ones((128, NTC, HPC, 2), ml_dtypes.bfloat16),
            "zrd": np.zeros((65, TQ), np.float32),
        })
    return in_maps


def assemble_output(results, n_cores=8, b_=B, t_=T, c_=C):
    """Sum per-core partial out^T over head groups, transpose back."""
    out = np.zeros((b_, t_, c_), np.float32)
    per_b = n_cores // b_
    for b in range(b_):
        acc = np.zeros((c_, t_), np.float32)
        for i in range(per_b):
            acc += results[b * per_b + i]["out_t"].astype(np.float32)
        out[b] = acc.T
    return out


_NC_CACHE = {}


def kernel(x, W_attn, W_proj, cache_k, cache_v):
    x = np.asarray(x, np.float32)
    W_attn = np.asarray(W_attn, np.float32)
    W_proj = np.asarray(W_proj, np.float32)
    cache_k = np.asarray(cache_k, np.float32)
    cache_v = np.asarray(cache_v, np.float32)
    if "nc" not in _NC_CACHE:
        _NC_CACHE["nc"] = build_kernel()
    nc = _NC_CACHE["nc"]
    in_maps = make_in_maps(x, W_attn, W_proj, cache_k, cache_v)
    res = run_bass_kernel_spmd(nc, in_maps, list(range(8)))
    return assemble_output(res.results)
